# revision 30
# baseline (speedup 1.0000x reference)
"""Trainium2 Bass kernel for nn_Attention (linear attention, no softmax).

Key identity: without softmax, (Q K^T) V = Q (K^T V), so the whole block
collapses to per-batch [C,C] matrices:
    S    = xs^T xs                     [C,C]   (xs = [L,C] tokens)
    At_h = Wk_h^T Wq_h  (= A_h^T)      [C,C]   batch-independent -> host-folded
    B_h  = Wv_h^T Wo_h^T               [C,C]   batch-independent -> host-folded
    Tt_h = S At_h   (= (A_h S)^T)      [C,C]
    G    = sum_h Tt_h^T B_h            [C,C]
    out  = (G^T X) + bias              [C,L]   (X = xs^T, the native x layout)

Sharding: data-parallel over batch, 2 batches per core across 8 cores.
bf16 on the wire and in the PE (f32 PSUM accumulate).

Perf notes (from trace):
- The PE clock ramps ~4.9us after first tensor-engine activity (1.2 ->
  2.4 GHz); 32 dummy N=128 matmuls at program start warm it while the
  input DMA lands, so the real matmul stream runs at full rate.
- DMA triggers (DIRECT2D ~650ns each) are issued on both HWDGE rings
  (sync + scalar) in parallel to start transfers sooner.
- Output is written as whole [P, L] tiles (2KB/partition descriptors);
  1KB descriptors halve per-engine DMA throughput. The final chunk is
  drained in halves so its DMA starts right after the first bias-add.
- Per (b, m) chunk the two bias-adds run on scalar (activation) and
  vector (tensor_scalar_add) in parallel.
"""

import numpy as np

P = 128
B_FULL, C, W, H = 16, 256, 32, 32
L = W * H  # 1024
NH = 4
NCORES = 8
BPC = B_FULL // NCORES  # batches per core = 2
CT = C // P   # 2 c-tiles
LT = L // P   # 8 L-tiles
NZ = L // 512  # 2 output column chunks

_CACHE = {}


def _np_bf16():
    import ml_dtypes
    return ml_dtypes.bfloat16


def _build_program():
    import concourse.bacc as bacc
    import concourse.mybir as mybir
    import concourse.tile as tile

    f32 = mybir.dt.float32
    mmdt = mybir.dt.bfloat16

    nc = bacc.Bacc("TRN2", target_bir_lowering=False, debug=False)

    # All inputs host-packed to [128, free] partition-major layouts.
    xs_d = nc.dram_tensor("xs", [BPC, P, LT * C], mmdt, kind="ExternalInput").ap()
    at_d = nc.dram_tensor("at", [P, CT * NH * C], mmdt, kind="ExternalInput").ap()
    b_d = nc.dram_tensor("b", [P, CT * NH * C], mmdt, kind="ExternalInput").ap()
    x2d_d = nc.dram_tensor("x2d", [BPC, P, CT * L], mmdt, kind="ExternalInput").ap()
    wob_d = nc.dram_tensor("wob", [P, CT], f32, kind="ExternalInput").ap()
    out_d = nc.dram_tensor("out", [P, BPC * CT * L], mmdt, kind="ExternalOutput").ap()

    with tile.TileContext(nc) as tc:
        from contextlib import ExitStack

        with ExitStack() as ctx:
            const = ctx.enter_context(tc.tile_pool(name="const", bufs=1))
            work = ctx.enter_context(tc.tile_pool(name="work", bufs=1))
            zpool = ctx.enter_context(tc.tile_pool(name="zout", bufs=4))
            psum = ctx.enter_context(tc.tile_pool(name="psum", bufs=8, space="PSUM"))

            def mm(ps_ap, lhsT_ap, rhs_ap, start, stop):
                nc.tensor.matmul(ps_ap, lhsT_ap, rhs_ap, start=start, stop=stop)

            # ---- DMAs on both HWDGE rings (sync + scalar), ordered by first use
            xs_sb = [work.tile([P, LT * C], mmdt, tag=f"xs{b}", name=f"xs_sb{b}") for b in range(BPC)]
            at_sb = const.tile([P, CT * NH * C], mmdt, tag="at")
            b_sb = const.tile([P, CT * NH * C], mmdt, tag="b")
            x_sb = [work.tile([P, CT * L], mmdt, tag=f"x{b}", name=f"x_sb{b}") for b in range(BPC)]
            bias_sb = const.tile([P, CT], f32, tag="bias")

            # PE warmup: ramp the tensor-engine clock while input DMA lands
            wu_sb = const.tile([P, P], mmdt, tag="wu")
            wu_ps = psum.tile([P, 512], f32, tag="ps", name="wu_ps")
            nc.gpsimd.memset(wu_sb[:], 0.0)
            for i in range(32):
                nc.tensor.matmul(wu_ps[:, :P], wu_sb[:], wu_sb[:],
                                 start=True, stop=True)

            nc.sync.dma_start(xs_sb[0][:], xs_d[0])
            nc.scalar.dma_start(xs_sb[1][:], xs_d[1])
            nc.sync.dma_start(at_sb[:], at_d[:])
            nc.scalar.dma_start(b_sb[:], b_d[:])
            nc.sync.dma_start(x_sb[0][:], x2d_d[0])
            nc.scalar.dma_start(x_sb[1][:], x2d_d[1])
            nc.sync.dma_start(bias_sb[:], wob_d[:])

            def copy_halves(dst_lo, src_lo, dst_hi, src_hi):
                nc.any.tensor_copy(dst_lo, src_lo)
                nc.any.tensor_copy(dst_hi, src_hi)

            # ---- S = xs^T xs per batch (symmetric)
            s_sb = [work.tile([P, CT * C], mmdt, tag=f"s{b}", name=f"s_sb{b}") for b in range(BPC)]

            def s_stage(b):
                ps = psum.tile([P, 512], f32, tag="ps")
                for m in range(CT):
                    for lt in range(LT):
                        mm(ps[:, m * C:(m + 1) * C],
                           xs_sb[b][:, lt * C + m * P: lt * C + m * P + P],
                           xs_sb[b][:, lt * C:(lt + 1) * C],
                           lt == 0, lt == LT - 1)
                copy_halves(s_sb[b][:, :C], ps[:, :C], s_sb[b][:, C:], ps[:, C:])

            # ---- Tt_h = S At_h ; layout [P, m*NH*C] like at_sb
            tt_sb = [work.tile([P, CT * NH * C], mmdt, tag=f"tt{b}", name=f"tt_sb{b}") for b in range(BPC)]

            def tt_stage(b):
                for m in range(CT):
                    pss = [psum.tile([P, 512], f32, tag="ps", name=f"ps_tt{m}_{i}") for i in range(NH // 2)]
                    for kt in range(CT):
                        for hp in range(NH // 2):  # consecutive mms share lhsT
                            mm(pss[hp][:],
                               s_sb[b][:, kt * C + m * P: kt * C + m * P + P],
                               at_sb[:, (kt * NH + hp * 2) * C:(kt * NH + hp * 2 + 2) * C],
                               kt == 0, kt == CT - 1)
                    o = (m * NH) * C
                    copy_halves(tt_sb[b][:, o:o + 512], pss[0][:],
                                tt_sb[b][:, o + 512:o + 1024], pss[1][:])

            # ---- G = sum_h Tt_h^T B_h
            g_sb = [work.tile([P, CT * C], mmdt, tag=f"g{b}", name=f"g_sb{b}") for b in range(BPC)]

            def g_stage(b):
                ps = psum.tile([P, 512], f32, tag="ps")
                for m in range(CT):
                    i, n_acc = 0, NH * CT
                    for h in range(NH):
                        for kt in range(CT):
                            mm(ps[:, m * C:(m + 1) * C],
                               tt_sb[b][:, (kt * NH + h) * C + m * P:(kt * NH + h) * C + m * P + P],
                               b_sb[:, (kt * NH + h) * C:(kt * NH + h + 1) * C],
                               i == 0, i == n_acc - 1)
                            i += 1
                copy_halves(g_sb[b][:, :C], ps[:, :C], g_sb[b][:, C:], ps[:, C:])

            # ---- out = G^T X + bias ; whole [P, L] per (b, m) -> 2KB descriptors
            def z_stage(b):
                for m in range(CT):
                    pss = [psum.tile([P, 512], f32, tag="ps", name=f"ps_z{m}_{i}") for i in range(NZ)]
                    for kt in range(CT):
                        for nt in range(NZ):  # consecutive mms share lhsT
                            mm(pss[nt][:],
                               g_sb[b][:, kt * C + m * P: kt * C + m * P + P],
                               x_sb[b][:, kt * L + nt * 512: kt * L + (nt + 1) * 512],
                               kt == 0, kt == CT - 1)
                    zb = zpool.tile([P, L], mmdt, tag="z")
                    bias_ap = bias_sb[:, m:m + 1]
                    nc.scalar.activation(
                        zb[:, :512], pss[0][:],
                        mybir.ActivationFunctionType.Identity, bias=bias_ap)
                    nc.vector.tensor_scalar_add(zb[:, 512:], pss[1][:], bias_ap)
                    base = (b * CT + m) * L
                    if b == BPC - 1 and m == CT - 1:
                        # final chunk: drain halves as each bias-add lands
                        nc.sync.dma_start(out_d[:, base:base + 512], zb[:, :512])
                        nc.sync.dma_start(out_d[:, base + 512:base + L], zb[:, 512:])
                    else:
                        nc.sync.dma_start(out_d[:, base:base + L], zb[:])

            # ---- schedule: interleave batches to keep PE gap-free
            s_stage(0)
            s_stage(1)
            tt_stage(0)
            tt_stage(1)
            g_stage(0)
            g_stage(1)
            z_stage(0)
            z_stage(1)

    nc.compile()
    return nc


def _get_program():
    if "nc" not in _CACHE:
        _CACHE["nc"] = _build_program()
    return _CACHE["nc"]


def _pack_rows(a, tiles):
    """[tiles*P, F] row-major -> [P, tiles*F] partition-major."""
    tP, F = a.shape
    assert tP == tiles * P
    return np.ascontiguousarray(
        a.reshape(tiles, P, F).transpose(1, 0, 2).reshape(P, tiles * F))


def _pack_w(Wt, ndt):
    """[NH, C, C] -> [P, CT*NH*C]: dst[p, (m*NH+h)*C+j] = Wt[h, m*P+p, j]."""
    a = np.asarray(Wt, np.float32).reshape(NH, CT, P, C)
    return np.ascontiguousarray(
        a.transpose(2, 1, 0, 3).reshape(P, CT * NH * C)).astype(ndt)


def _prep_inputs(x, Wq, Wk, Wv, Wo_w, Wo_b):
    ndt = _np_bf16()
    x = np.asarray(x, dtype=np.float32)
    X = x.reshape(B_FULL, C, L)                                    # [b, C, L]
    XS = X.transpose(0, 2, 1)                                      # [b, L, C]
    Wq = np.asarray(Wq, np.float32)
    Wk = np.asarray(Wk, np.float32)
    Wv = np.asarray(Wv, np.float32)
    WoT = np.ascontiguousarray(np.asarray(Wo_w, np.float32).T).reshape(NH, C, C)

    # fold the batch-independent weight products on host
    At = np.einsum('hdc,hde->hce', Wk, Wq)   # At_h = Wk_h^T Wq_h
    Bm = np.einsum('hdc,hde->hce', Wv, WoT)  # B_h  = Wv_h^T WoT_h

    common = {
        "at": _pack_w(At, ndt), "b": _pack_w(Bm, ndt),
        "wob": np.ascontiguousarray(
            np.asarray(Wo_b, np.float32).reshape(CT, P).T),
    }
    in_maps = []
    for i in range(NCORES):
        bs = slice(i * BPC, (i + 1) * BPC)
        x2d_p = np.stack([_pack_rows(Xb, CT) for Xb in X[bs]]).astype(ndt)
        xs_p = np.stack([_pack_rows(Sb, LT) for Sb in XS[bs]]).astype(ndt)
        in_maps.append({"x2d": x2d_p, "xs": xs_p, **common})
    return in_maps


def _unpack_out(res_list):
    """per-core [P, BPC*CT*L] -> [B_FULL, C, W, H]"""
    out = np.empty((B_FULL, C, L), dtype=np.float32)
    for i in range(NCORES):
        o = np.asarray(res_list[i]["out"], dtype=np.float32).reshape(P, BPC, CT, L)
        for b in range(BPC):
            out[i * BPC + b] = o[:, b].transpose(1, 0, 2).reshape(C, L)
    return out.reshape(B_FULL, C, W, H)


def run_sharded(inputs, trace=False, trace_cores=None):
    """Run the SPMD kernel; returns (full_output, BassKernelResults)."""
    from concourse.bass_utils import run_bass_kernel_spmd

    in_maps = _prep_inputs(**inputs)
    nc = _get_program()
    res = run_bass_kernel_spmd(
        nc, in_maps, core_ids=list(range(NCORES)),
        trace=trace, trace_cores=trace_cores,
    )
    return _unpack_out(res.results), res


def kernel(x, Wq, Wk, Wv, Wo_w, Wo_b):
    out, _ = run_sharded(
        {"x": x, "Wq": Wq, "Wk": Wk, "Wv": Wv, "Wo_w": Wo_w, "Wo_b": Wo_b}
    )
    return out


# revision 31
# speedup vs baseline: 1.0086x; 1.0086x over previous
"""Trainium2 Bass kernel for nn_Attention (linear attention, no softmax).

Key identity: without softmax, (Q K^T) V = Q (K^T V), so the whole block
collapses to per-batch [C,C] matrices:
    S    = xs^T xs                     [C,C]   (xs = [L,C] tokens)
    At_h = Wk_h^T Wq_h  (= A_h^T)      [C,C]   batch-independent -> host-folded
    B_h  = Wv_h^T Wo_h^T               [C,C]   batch-independent -> host-folded
    Tt_h = S At_h   (= (A_h S)^T)      [C,C]
    G    = sum_h Tt_h^T B_h            [C,C]
    out  = (G^T X) + bias              [C,L]   (X = xs^T, the native x layout)

Sharding: data-parallel over batch, 2 batches per core across 8 cores.
bf16 on the wire and in the PE (f32 PSUM accumulate).

Perf notes (from trace):
- The PE clock ramps ~4.9us after first tensor-engine activity (1.2 ->
  2.4 GHz); 32 dummy N=128 matmuls at program start warm it while the
  input DMA lands, so the real matmul stream runs at full rate.
- DMA triggers (DIRECT2D ~650ns each) are issued on both HWDGE rings
  (sync + scalar) in parallel to start transfers sooner.
- Output is written as whole [P, L] tiles (2KB/partition descriptors);
  1KB descriptors halve per-engine DMA throughput. The final chunk is
  drained in halves so its DMA starts right after the first bias-add.
- Per (b, m) chunk the two bias-adds run on scalar (activation) and
  vector (tensor_scalar_add) in parallel.
"""

import numpy as np

P = 128
B_FULL, C, W, H = 16, 256, 32, 32
L = W * H  # 1024
NH = 4
NCORES = 8
BPC = B_FULL // NCORES  # batches per core = 2
CT = C // P   # 2 c-tiles
LT = L // P   # 8 L-tiles
NZ = L // 512  # 2 output column chunks

_CACHE = {}


def _np_bf16():
    import ml_dtypes
    return ml_dtypes.bfloat16


def _build_program():
    import concourse.bacc as bacc
    import concourse.mybir as mybir
    import concourse.tile as tile

    f32 = mybir.dt.float32
    mmdt = mybir.dt.bfloat16

    nc = bacc.Bacc("TRN2", target_bir_lowering=False, debug=False)

    # All inputs host-packed to [128, free] partition-major layouts.
    xs_d = nc.dram_tensor("xs", [BPC, P, LT * C], mmdt, kind="ExternalInput").ap()
    at_d = nc.dram_tensor("at", [P, CT * NH * C], mmdt, kind="ExternalInput").ap()
    b_d = nc.dram_tensor("b", [P, CT * NH * C], mmdt, kind="ExternalInput").ap()
    x2d_d = nc.dram_tensor("x2d", [BPC, P, CT * L], mmdt, kind="ExternalInput").ap()
    wob_d = nc.dram_tensor("wob", [P, CT], f32, kind="ExternalInput").ap()
    out_d = nc.dram_tensor("out", [P, BPC * CT * L], mmdt, kind="ExternalOutput").ap()

    with tile.TileContext(nc) as tc:
        from contextlib import ExitStack

        with ExitStack() as ctx:
            const = ctx.enter_context(tc.tile_pool(name="const", bufs=1))
            work = ctx.enter_context(tc.tile_pool(name="work", bufs=1))
            zpool = ctx.enter_context(tc.tile_pool(name="zout", bufs=4))
            psum = ctx.enter_context(tc.tile_pool(name="psum", bufs=8, space="PSUM"))

            def mm(ps_ap, lhsT_ap, rhs_ap, start, stop):
                nc.tensor.matmul(ps_ap, lhsT_ap, rhs_ap, start=start, stop=stop)

            # ---- DMAs on both HWDGE rings (sync + scalar), ordered by first use
            xs_sb = [work.tile([P, LT * C], mmdt, tag=f"xs{b}", name=f"xs_sb{b}") for b in range(BPC)]
            at_sb = const.tile([P, CT * NH * C], mmdt, tag="at")
            b_sb = const.tile([P, CT * NH * C], mmdt, tag="b")
            x_sb = [work.tile([P, CT * L], mmdt, tag=f"x{b}", name=f"x_sb{b}") for b in range(BPC)]
            bias_sb = const.tile([P, CT], f32, tag="bias")

            # PE warmup: ramp the tensor-engine clock while input DMA lands
            wu_sb = const.tile([P, P], mmdt, tag="wu")
            wu_ps = psum.tile([P, 512], f32, tag="ps", name="wu_ps")
            nc.vector.memset(wu_sb[:], 0.0)
            for i in range(32):
                nc.tensor.matmul(wu_ps[:, :P], wu_sb[:], wu_sb[:],
                                 start=True, stop=True)

            nc.sync.dma_start(xs_sb[0][:], xs_d[0])
            nc.scalar.dma_start(xs_sb[1][:], xs_d[1])
            nc.sync.dma_start(at_sb[:], at_d[:])
            nc.scalar.dma_start(b_sb[:], b_d[:])
            nc.sync.dma_start(x_sb[0][:], x2d_d[0])
            nc.scalar.dma_start(x_sb[1][:], x2d_d[1])
            nc.sync.dma_start(bias_sb[:], wob_d[:])

            def copy_halves(dst_lo, src_lo, dst_hi, src_hi):
                nc.any.tensor_copy(dst_lo, src_lo)
                nc.any.tensor_copy(dst_hi, src_hi)

            # ---- S = xs^T xs per batch (symmetric)
            s_sb = [work.tile([P, CT * C], mmdt, tag=f"s{b}", name=f"s_sb{b}") for b in range(BPC)]

            def s_stage(b):
                ps = psum.tile([P, 512], f32, tag="ps")
                for m in range(CT):
                    for lt in range(LT):
                        mm(ps[:, m * C:(m + 1) * C],
                           xs_sb[b][:, lt * C + m * P: lt * C + m * P + P],
                           xs_sb[b][:, lt * C:(lt + 1) * C],
                           lt == 0, lt == LT - 1)
                copy_halves(s_sb[b][:, :C], ps[:, :C], s_sb[b][:, C:], ps[:, C:])

            # ---- Tt_h = S At_h ; layout [P, m*NH*C] like at_sb
            tt_sb = [work.tile([P, CT * NH * C], mmdt, tag=f"tt{b}", name=f"tt_sb{b}") for b in range(BPC)]

            def tt_stage(b):
                for m in range(CT):
                    pss = [psum.tile([P, 512], f32, tag="ps", name=f"ps_tt{m}_{i}") for i in range(NH // 2)]
                    for kt in range(CT):
                        for hp in range(NH // 2):  # consecutive mms share lhsT
                            mm(pss[hp][:],
                               s_sb[b][:, kt * C + m * P: kt * C + m * P + P],
                               at_sb[:, (kt * NH + hp * 2) * C:(kt * NH + hp * 2 + 2) * C],
                               kt == 0, kt == CT - 1)
                    o = (m * NH) * C
                    copy_halves(tt_sb[b][:, o:o + 512], pss[0][:],
                                tt_sb[b][:, o + 512:o + 1024], pss[1][:])

            # ---- G = sum_h Tt_h^T B_h
            g_sb = [work.tile([P, CT * C], mmdt, tag=f"g{b}", name=f"g_sb{b}") for b in range(BPC)]

            def g_stage(b):
                ps = psum.tile([P, 512], f32, tag="ps")
                for m in range(CT):
                    i, n_acc = 0, NH * CT
                    for h in range(NH):
                        for kt in range(CT):
                            mm(ps[:, m * C:(m + 1) * C],
                               tt_sb[b][:, (kt * NH + h) * C + m * P:(kt * NH + h) * C + m * P + P],
                               b_sb[:, (kt * NH + h) * C:(kt * NH + h + 1) * C],
                               i == 0, i == n_acc - 1)
                            i += 1
                copy_halves(g_sb[b][:, :C], ps[:, :C], g_sb[b][:, C:], ps[:, C:])

            # ---- out = G^T X + bias ; whole [P, L] per (b, m) -> 2KB descriptors
            def z_stage(b):
                for m in range(CT):
                    pss = [psum.tile([P, 512], f32, tag="ps", name=f"ps_z{m}_{i}") for i in range(NZ)]
                    for kt in range(CT):
                        for nt in range(NZ):  # consecutive mms share lhsT
                            mm(pss[nt][:],
                               g_sb[b][:, kt * C + m * P: kt * C + m * P + P],
                               x_sb[b][:, kt * L + nt * 512: kt * L + (nt + 1) * 512],
                               kt == 0, kt == CT - 1)
                    zb = zpool.tile([P, L], mmdt, tag="z")
                    bias_ap = bias_sb[:, m:m + 1]
                    nc.scalar.activation(
                        zb[:, :512], pss[0][:],
                        mybir.ActivationFunctionType.Identity, bias=bias_ap)
                    nc.vector.tensor_scalar_add(zb[:, 512:], pss[1][:], bias_ap)
                    base = (b * CT + m) * L
                    # last batch: chunks on different rings so their
                    # descriptor generations run in parallel
                    eng = nc.scalar if (b == BPC - 1 and m == CT - 1) else nc.sync
                    eng.dma_start(out_d[:, base:base + L], zb[:])

            # ---- schedule: interleave batches to keep PE gap-free
            s_stage(0)
            s_stage(1)
            tt_stage(0)
            tt_stage(1)
            g_stage(0)
            g_stage(1)
            z_stage(0)
            z_stage(1)

    nc.compile()
    return nc


def _get_program():
    if "nc" not in _CACHE:
        _CACHE["nc"] = _build_program()
    return _CACHE["nc"]


def _pack_rows(a, tiles):
    """[tiles*P, F] row-major -> [P, tiles*F] partition-major."""
    tP, F = a.shape
    assert tP == tiles * P
    return np.ascontiguousarray(
        a.reshape(tiles, P, F).transpose(1, 0, 2).reshape(P, tiles * F))


def _pack_w(Wt, ndt):
    """[NH, C, C] -> [P, CT*NH*C]: dst[p, (m*NH+h)*C+j] = Wt[h, m*P+p, j]."""
    a = np.asarray(Wt, np.float32).reshape(NH, CT, P, C)
    return np.ascontiguousarray(
        a.transpose(2, 1, 0, 3).reshape(P, CT * NH * C)).astype(ndt)


def _prep_inputs(x, Wq, Wk, Wv, Wo_w, Wo_b):
    ndt = _np_bf16()
    x = np.asarray(x, dtype=np.float32)
    X = x.reshape(B_FULL, C, L)                                    # [b, C, L]
    XS = X.transpose(0, 2, 1)                                      # [b, L, C]
    Wq = np.asarray(Wq, np.float32)
    Wk = np.asarray(Wk, np.float32)
    Wv = np.asarray(Wv, np.float32)
    WoT = np.ascontiguousarray(np.asarray(Wo_w, np.float32).T).reshape(NH, C, C)

    # fold the batch-independent weight products on host
    At = np.einsum('hdc,hde->hce', Wk, Wq)   # At_h = Wk_h^T Wq_h
    Bm = np.einsum('hdc,hde->hce', Wv, WoT)  # B_h  = Wv_h^T WoT_h

    common = {
        "at": _pack_w(At, ndt), "b": _pack_w(Bm, ndt),
        "wob": np.ascontiguousarray(
            np.asarray(Wo_b, np.float32).reshape(CT, P).T),
    }
    in_maps = []
    for i in range(NCORES):
        bs = slice(i * BPC, (i + 1) * BPC)
        x2d_p = np.stack([_pack_rows(Xb, CT) for Xb in X[bs]]).astype(ndt)
        xs_p = np.stack([_pack_rows(Sb, LT) for Sb in XS[bs]]).astype(ndt)
        in_maps.append({"x2d": x2d_p, "xs": xs_p, **common})
    return in_maps


def _unpack_out(res_list):
    """per-core [P, BPC*CT*L] -> [B_FULL, C, W, H]"""
    out = np.empty((B_FULL, C, L), dtype=np.float32)
    for i in range(NCORES):
        o = np.asarray(res_list[i]["out"], dtype=np.float32).reshape(P, BPC, CT, L)
        for b in range(BPC):
            out[i * BPC + b] = o[:, b].transpose(1, 0, 2).reshape(C, L)
    return out.reshape(B_FULL, C, W, H)


def run_sharded(inputs, trace=False, trace_cores=None):
    """Run the SPMD kernel; returns (full_output, BassKernelResults)."""
    from concourse.bass_utils import run_bass_kernel_spmd

    in_maps = _prep_inputs(**inputs)
    nc = _get_program()
    res = run_bass_kernel_spmd(
        nc, in_maps, core_ids=list(range(NCORES)),
        trace=trace, trace_cores=trace_cores,
    )
    return _unpack_out(res.results), res


def kernel(x, Wq, Wk, Wv, Wo_w, Wo_b):
    out, _ = run_sharded(
        {"x": x, "Wq": Wq, "Wk": Wk, "Wv": Wv, "Wo_w": Wo_w, "Wo_b": Wo_b}
    )
    return out
